# revision 21
# baseline (speedup 1.0000x reference)
"""Self-attention (CrossAttention module with q=k=v=x) kernel for Trainium2.

Problem: x [B=4, N=4096, H=256] fp32; Wq/Wk/Wv [256,256], bq/bk/bv [256].
  q = x@Wq.T+bq ; k = x@Wk.T+bk ; v = x@Wv.T+bv
  out = softmax(q@k.T) @ v          (no 1/sqrt(d) scaling)

Sharding: 8 cores = batch (4) x query-halves (2). Each core holds the full
K/V sequence for its batch element and 2048 query rows.

Scores algebra: q_i.k_j = x_i (Wq^T Wk) x_j^T + u_i + w_j + c where u_i, c
are constant per softmax row (dropped -- softmax-invariant) and
w_j = x_j . (Wk^T bq) is applied as the exp() bias on-device.  So the
device only needs qT = (x_half @ Wq^T Wk)^T, the raw keys xT = x^T, and
V = x @ Wv^T -- the small O(N H^2) projections are folded on the host
(f64), leaving the O(N^2 H) attention as pure device work.

Per-core device schedule (PE-bound: ~131k cycles of fp16 scores matmuls +
~132k cycles of bf16 AV matmuls):
  - scores are computed TRANSPOSED: S_T[j, i] = sum_o xT[o,j] * qT[o,i]
    (key chunk stationary, qT moving), so exp(S_T) feeds the AV matmul
    as the stationary operand with no on-chip transpose.  Query blocks
    are processed in pairs sharing each key-chunk stationary load.
  - exp(S) runs bf16 (fp32-like exponent range -- no row-max pass needed);
    V carries two trailing ones columns so the softmax denominator comes
    free off the AV matmul; normalize = DVE reciprocal + per-partition
    broadcast multiply; v-bias is added on the host after gathering.
  - a junk-matmul warmup on a memset tile (no DMA dependency) ramps the
    HAM clock gate to 2.4 GHz while inputs stream in, and a dummy exp()
    right after it pulls the ~2.7us ACT_TABLE_LOAD out of the critical
    path.
"""

import sys

import numpy as np

if "/opt/trn_rl_repo" not in sys.path:
    sys.path.insert(0, "/opt/trn_rl_repo")

B, N, H = 4, 4096, 256
P = 128
NQ = N // 2          # query rows per core
JC = N // P          # key chunks (32)
IBLK = 512           # query block
ICH = IBLK // P      # query sub-chunks per block (4)
NPAIR = NQ // (2 * IBLK)  # block pairs per core (2)


def build_nc(salt=0):
    import concourse.mybir as mybir
    import concourse.tile as tile
    from concourse import bacc

    f32 = mybir.dt.float32
    FR = mybir.dt.float16    # scores matmul dtype (11-bit mantissa)
    AVT = mybir.dt.bfloat16  # exp(S) and V dtype: needs fp32-like range
    Exp = mybir.ActivationFunctionType.Exp

    nc = bacc.Bacc("TRN2", target_bir_lowering=False, debug=False)

    xT_d = nc.dram_tensor("xT", [H, N], FR, kind="ExternalInput").ap()
    qT_d = nc.dram_tensor("qT", [H, NQ], FR, kind="ExternalInput").ap()
    v_d = nc.dram_tensor("v", [N, H + 2], AVT, kind="ExternalInput").ap()
    wsc_d = nc.dram_tensor("wsc", [P, JC], f32, kind="ExternalInput").ap()
    att_d = nc.dram_tensor("att", [NQ, H], AVT, kind="ExternalOutput").ap()
    warm_d = nc.dram_tensor("warm", [P, 4], f32, kind="ExternalOutput").ap()
    gate_d = nc.dram_tensor("gate", [P, 2], FR, kind="ExternalOutput").ap()

    with tile.TileContext(nc) as tc:
        with tc.tile_pool(name="io", bufs=1) as io, \
             tc.tile_pool(name="kqv", bufs=1) as kqv, \
             tc.tile_pool(name="expp", bufs=JC + 8) as expp, \
             tc.tile_pool(name="op", bufs=4) as op, \
             tc.tile_pool(name="psmm", bufs=3, space="PSUM") as psmm, \
             tc.tile_pool(name="psatt", bufs=5, space="PSUM") as psatt:

            # ---- input loads, emitted first so the sync queue dispatches
            # the doorbells as early as possible.  The first-needed chunks
            # are split fine (64KB) so no single queue serializes the
            # critical path; later chunks are larger for DMA efficiency.
            xt = [io.tile([P, N], FR, tag=f"xt{h}", name=f"xt{h}") for h in range(2)]
            qt = [io.tile([P, NQ], FR, tag=f"qt{h}", name=f"qt{h}") for h in range(2)]
            vt = [kqv.tile([P, H + 2], AVT, tag=f"v{j}", name=f"v{j}") for j in range(JC)]
            wsc = io.tile([P, JC], f32, tag="wsc", name="wsc")

            def ld_h(sb_tiles, dr, h, cols):
                hs = slice(h * P, (h + 1) * P)
                nc.sync.dma_start(sb_tiles[h][:, cols], dr[hs, cols])

            def ld_cols(sb_tiles, dr, cols):
                for h in range(2):
                    ld_h(sb_tiles, dr, h, cols)

            def ld_v(j):
                nc.sync.dma_start(vt[j][:], v_d[j * P:(j + 1) * P, :])

            # need-order; each chain ahead of the critical set costs
            # ~0.6us of serial sync-sequencer dispatch, so the two chains
            # the very first matmul needs go absolutely first.
            ld_h(qt, qT_d, 0, slice(0, 512))
            ld_h(xt, xT_d, 0, slice(0, 256))
            ld_h(qt, qT_d, 1, slice(0, 512))
            ld_h(xt, xT_d, 1, slice(0, 256))
            nc.sync.dma_start(wsc[:], wsc_d[:])
            ld_cols(qt, qT_d, slice(512, 1024))
            ld_v(0)
            ld_v(1)
            ld_cols(xt, xT_d, slice(256, 512))
            ld_v(2)
            ld_v(3)
            ld_cols(xt, xT_d, slice(512, 1024))
            ld_v(4)
            ld_v(5)
            ld_cols(xt, xT_d, slice(1024, 2048))
            for j in range(6, 10):
                ld_v(j)
            ld_cols(xt, xT_d, slice(2048, 3072))
            for j in range(10, 18):
                ld_v(j)
            ld_cols(xt, xT_d, slice(3072, 4096))
            for j in range(18, 26):
                ld_v(j)
            ld_cols(qt, qT_d, slice(1024, 2048))
            for j in range(26, 32):
                ld_v(j)

            # ---- PE warm-up + ACT table preload (no DMA dependencies;
            # gpsimd memset because that engine comes alive first) ----
            wrm = io.tile([P, 256], FR, tag="wrm", name="wrm")
            nc.gpsimd.memset(wrm[:], 0.5)
            wex = op.tile([P, 2], f32, tag="wex", name="wex")
            nc.scalar.activation(wex[:], wrm[:, 0:2], Exp)
            wps = psmm.tile([P, 256], f32, tag="mm", name="wps")
            nwarm = 13 + salt
            for r in range(nwarm):
                nc.tensor.matmul(wps[:, 0:P], wrm[:, 0:P], wrm[:, 0:P],
                                 start=(r == 0), stop=(r == nwarm - 1))
            # read wps out immediately: its PSUM buffer (tag "mm") is
            # recycled by the scores tiles below.  The flush keeps the
            # junk matmuls + dummy exp alive through DCE.
            wsb = op.tile([P, 4], f32, tag="wsb", name="wsb")
            nc.vector.tensor_copy(wsb[:, 0:2], wps[:, 0:2])
            nc.vector.tensor_copy(wsb[:, 2:4], wex[:])
            nc.sync.dma_start(warm_d[:], wsb[:])

            # ---- attention block pairs ----
            # The scores matmuls for both blocks of a pair share each
            # key-chunk stationary load.  Block b0's AV runs inline per
            # key-chunk; block b1's exp(S) tiles are buffered in SBUF and
            # consumed in a second AV sweep (PSUM can only hold one
            # block's accumulators plus the rotating scores tiles).
            def normalize_one(att_tile, blk, ic):
                rec = op.tile([P, 1], f32, tag="rec", name="rec")
                nc.vector.reciprocal(rec[:], att_tile[:, H:H + 1])
                ao = op.tile([P, H], AVT, tag="ao", name="ao")
                nc.vector.tensor_scalar_mul(ao[:], att_tile[:, 0:H], rec[:])
                r0 = blk * IBLK + ic * P
                nc.sync.dma_start(att_d[r0:r0 + P, :], ao[:])

            for pair in range(NPAIR):
                bss = [slice((2 * pair + b) * IBLK, (2 * pair + b + 1) * IBLK)
                       for b in range(2)]
                att_ps = [psatt.tile([P, H + 2], f32, tag="att", name="attps")
                          for _ in range(ICH)]
                exs = [[], []]
                for jc in range(JC):
                    jcs = slice(jc * P, (jc + 1) * P)
                    scs = [psmm.tile([P, IBLK], f32, tag="mm", name=f"sc{b}")
                           for b in range(2)]
                    if pair == 0 and jc < 4:
                        # block-major: b0 only needs the first qt chunks,
                        # which land ~1us before b1's
                        for b in range(2):
                            for oc in range(2):
                                nc.tensor.matmul(scs[b][:], xt[oc][:, jcs],
                                                 qt[oc][:, bss[b]],
                                                 start=(oc == 0), stop=(oc == 1))
                    else:
                        for oc in range(2):
                            for b in range(2):
                                nc.tensor.matmul(scs[b][:], xt[oc][:, jcs],
                                                 qt[oc][:, bss[b]],
                                                 start=(oc == 0), stop=(oc == 1))
                    for b in range(2):
                        ex = expp.tile([P, IBLK], AVT, tag="ex", name=f"ex{b}")
                        nc.scalar.activation(ex[:], scs[b][:], Exp,
                                             bias=wsc[:, jc:jc + 1])
                        exs[b].append(ex)
                    for ic in range(ICH):
                        ics = slice(ic * P, (ic + 1) * P)
                        nc.tensor.matmul(att_ps[ic][:], exs[0][jc][:, ics],
                                         vt[jc][:],
                                         start=(jc == 0), stop=(jc == JC - 1))
                for ic in range(ICH):
                    normalize_one(att_ps[ic], 2 * pair, ic)
                last = (pair == NPAIR - 1)
                for ic in range(ICH):
                    ics = slice(ic * P, (ic + 1) * P)
                    if last and ic == ICH - 1:
                        # final accumulator: split by V columns across two
                        # PSUM banks so the high half's normalize + DMA-out
                        # overlaps the low half's AV sweep.
                        pa = psatt.tile([P, H - P + 2], f32, tag="att",
                                        name="attpa")
                        pb = psatt.tile([P, P], f32, tag="att", name="attpb")
                        for jc in range(JC):
                            nc.tensor.matmul(pa[:], exs[1][jc][:, ics],
                                             vt[jc][:, P:H + 2],
                                             start=(jc == 0),
                                             stop=(jc == JC - 1))
                        rec = op.tile([P, 1], f32, tag="rec", name="rec")
                        nc.vector.reciprocal(rec[:], pa[:, H - P:H - P + 1])
                        ah = op.tile([P, H - P], AVT, tag="ao", name="ah")
                        nc.vector.tensor_scalar_mul(ah[:], pa[:, 0:H - P],
                                                    rec[:])
                        r0 = (2 * pair + 1) * IBLK + ic * P
                        nc.sync.dma_start(att_d[r0:r0 + P, P:H], ah[:])
                        for jc in range(JC):
                            nc.tensor.matmul(pb[:], exs[1][jc][:, ics],
                                             vt[jc][:, 0:P],
                                             start=(jc == 0),
                                             stop=(jc == JC - 1))
                        al = op.tile([P, P], AVT, tag="ao", name="al")
                        nc.vector.tensor_scalar_mul(al[:], pb[:], rec[:])
                        nc.sync.dma_start(att_d[r0:r0 + P, 0:P], al[:])
                    else:
                        pf = psatt.tile([P, H + 2], f32, tag="att",
                                        name="attpsb")
                        for jc in range(JC):
                            nc.tensor.matmul(pf[:], exs[1][jc][:, ics],
                                             vt[jc][:],
                                             start=(jc == 0),
                                             stop=(jc == JC - 1))
                        normalize_one(pf, 2 * pair + 1, ic)

    nc.compile()
    return nc


_NC_CACHE = {}


def _get_nc():
    if "nc" not in _NC_CACHE:
        _NC_CACHE["nc"] = build_nc()
    return _NC_CACHE["nc"]


def _make_in_maps(x, Wq, bq, Wk, bk, Wv):
    import ml_dtypes

    bf16 = ml_dtypes.bfloat16
    A = Wq.T.astype(np.float64) @ Wk.astype(np.float64)
    wkbq = Wk.T.astype(np.float64) @ bq.astype(np.float64)
    in_maps = []
    for b in range(B):
        xb = x[b].astype(np.float64)
        wsc_b = np.ascontiguousarray(
            (xb @ wkbq).astype(np.float32).reshape(JC, P).T)
        v_b = np.empty((N, H + 2), dtype=bf16)
        v_b[:, 0:H] = (xb @ Wv.T.astype(np.float64)).astype(bf16)
        v_b[:, H:] = np.ones((N, 2), dtype=bf16)
        v_b = np.ascontiguousarray(v_b)
        xT_b = np.ascontiguousarray(x[b].astype(np.float16).T)
        q_b = (xb @ A).astype(np.float16)
        for half in range(2):
            qT = np.ascontiguousarray(q_b[half * NQ:(half + 1) * NQ, :].T)
            in_maps.append({"xT": xT_b, "qT": qT, "v": v_b, "wsc": wsc_b})
    return in_maps


def _run(inputs, trace=False):
    from concourse.bass_utils import run_bass_kernel_spmd

    x = np.asarray(inputs["x"], dtype=np.float32)
    Wq = np.asarray(inputs["Wq"], dtype=np.float32)
    bq = np.asarray(inputs["bq"], dtype=np.float32)
    Wk = np.asarray(inputs["Wk"], dtype=np.float32)
    bk = np.asarray(inputs["bk"], dtype=np.float32)
    Wv = np.asarray(inputs["Wv"], dtype=np.float32)
    bv = np.asarray(inputs["bv"], dtype=np.float32)

    in_maps = _make_in_maps(x, Wq, bq, Wk, bk, Wv)
    # The device occasionally wedges on the first execution of a fresh
    # NEFF (NRT_EXEC_UNIT_UNRECOVERABLE); a retry with a slightly
    # perturbed program (different walrus schedule) recovers.
    last_exc = None
    for attempt in range(3):
        try:
            nc = _get_nc() if attempt == 0 else build_nc(salt=attempt)
            res = run_bass_kernel_spmd(nc, in_maps, list(range(8)), trace=trace)
            break
        except Exception as e:  # noqa: BLE001
            last_exc = e
            import os as _os
            import time as _time
            _os.environ["NEURON_RT_RESET_CORES"] = "1"
            _time.sleep(3)
    else:
        raise last_exc

    out = np.empty((B, N, H), dtype=np.float32)
    for c in range(8):
        b, half = c // 2, c % 2
        out[b, half * NQ:(half + 1) * NQ, :] = \
            res.results[c]["att"].astype(np.float32) + bv
    return out, res


def kernel(**inputs) -> np.ndarray:
    out, _ = _run(inputs, trace=False)
    return out


# revision 23
# speedup vs baseline: 1.0134x; 1.0134x over previous
"""Self-attention (CrossAttention module with q=k=v=x) kernel for Trainium2.

Problem: x [B=4, N=4096, H=256] fp32; Wq/Wk/Wv [256,256], bq/bk/bv [256].
  q = x@Wq.T+bq ; k = x@Wk.T+bk ; v = x@Wv.T+bv
  out = softmax(q@k.T) @ v          (no 1/sqrt(d) scaling)

Sharding: 8 cores = batch (4) x query-halves (2). Each core holds the full
K/V sequence for its batch element and 2048 query rows.

Scores algebra: q_i.k_j = x_i (Wq^T Wk) x_j^T + u_i + w_j + c where u_i, c
are constant per softmax row (dropped -- softmax-invariant) and
w_j = x_j . (Wk^T bq) is applied as the exp() bias on-device.  So the
device only needs qT = (x_half @ Wq^T Wk)^T, the raw keys xT = x^T, and
V = x @ Wv^T -- the small O(N H^2) projections are folded on the host
(f64), leaving the O(N^2 H) attention as pure device work.

Per-core device schedule (PE-bound: ~131k cycles of fp16 scores matmuls +
~132k cycles of bf16 AV matmuls):
  - scores are computed TRANSPOSED: S_T[j, i] = sum_o xT[o,j] * qT[o,i]
    (key chunk stationary, qT moving), so exp(S_T) feeds the AV matmul
    as the stationary operand with no on-chip transpose.  Query blocks
    are processed in pairs sharing each key-chunk stationary load.
  - exp(S) runs bf16 (fp32-like exponent range -- no row-max pass needed);
    V carries two trailing ones columns so the softmax denominator comes
    free off the AV matmul; normalize = DVE reciprocal + per-partition
    broadcast multiply; v-bias is added on the host after gathering.
  - a junk-matmul warmup on a memset tile (no DMA dependency) ramps the
    HAM clock gate to 2.4 GHz while inputs stream in, and a dummy exp()
    right after it pulls the ~2.7us ACT_TABLE_LOAD out of the critical
    path.
"""

import sys

import numpy as np

if "/opt/trn_rl_repo" not in sys.path:
    sys.path.insert(0, "/opt/trn_rl_repo")

B, N, H = 4, 4096, 256
P = 128
NQ = N // 2          # query rows per core
JC = N // P          # key chunks (32)
IBLK = 512           # query block
ICH = IBLK // P      # query sub-chunks per block (4)
NPAIR = NQ // (2 * IBLK)  # block pairs per core (2)


def build_nc(salt=0):
    import concourse.mybir as mybir
    import concourse.tile as tile
    from concourse import bacc

    f32 = mybir.dt.float32
    FR = mybir.dt.float16    # scores matmul dtype (11-bit mantissa)
    AVT = mybir.dt.bfloat16  # exp(S) and V dtype: needs fp32-like range
    Exp = mybir.ActivationFunctionType.Exp

    nc = bacc.Bacc("TRN2", target_bir_lowering=False, debug=False)

    xT_d = nc.dram_tensor("xT", [H, N], FR, kind="ExternalInput").ap()
    qT_d = nc.dram_tensor("qT", [H, NQ], FR, kind="ExternalInput").ap()
    v_d = nc.dram_tensor("v", [N, H + 2], AVT, kind="ExternalInput").ap()
    wsc_d = nc.dram_tensor("wsc", [P, JC], f32, kind="ExternalInput").ap()
    att_d = nc.dram_tensor("att", [NQ, H], AVT, kind="ExternalOutput").ap()
    warm_d = nc.dram_tensor("warm", [P, 4], f32, kind="ExternalOutput").ap()
    gate_d = nc.dram_tensor("gate", [P, 2], FR, kind="ExternalOutput").ap()

    with tile.TileContext(nc) as tc:
        with tc.tile_pool(name="io", bufs=1) as io, \
             tc.tile_pool(name="kqv", bufs=1) as kqv, \
             tc.tile_pool(name="expp", bufs=JC + 8) as expp, \
             tc.tile_pool(name="op", bufs=4) as op, \
             tc.tile_pool(name="psmm", bufs=3, space="PSUM") as psmm, \
             tc.tile_pool(name="psatt", bufs=5, space="PSUM") as psatt:

            # ---- input loads, emitted first so the sync queue dispatches
            # the doorbells as early as possible.  The first-needed chunks
            # are split fine (64KB) so no single queue serializes the
            # critical path; later chunks are larger for DMA efficiency.
            xt = [io.tile([P, N], FR, tag=f"xt{h}", name=f"xt{h}") for h in range(2)]
            qt = [io.tile([P, NQ], FR, tag=f"qt{h}", name=f"qt{h}") for h in range(2)]
            vt = [kqv.tile([P, H + 2], AVT, tag=f"v{j}", name=f"v{j}") for j in range(JC)]
            wsc = io.tile([P, JC], f32, tag="wsc", name="wsc")

            def ld_h(sb_tiles, dr, h, cols):
                hs = slice(h * P, (h + 1) * P)
                nc.sync.dma_start(sb_tiles[h][:, cols], dr[hs, cols])

            def ld_cols(sb_tiles, dr, cols):
                for h in range(2):
                    ld_h(sb_tiles, dr, h, cols)

            def ld_v(j):
                nc.sync.dma_start(vt[j][:], v_d[j * P:(j + 1) * P, :])

            # The critical head chains ride the scalar engine's DGE ring
            # (qActDynamicHW): its preamble finishes earlier and nothing
            # competes there, so the first scores matmuls unblock ~3us
            # sooner than via the (busy) sync ring.
            hs1 = slice(P, 2 * P)
            nc.scalar.dma_start(qt[0][:, 0:512], qT_d[0:P, 0:512])
            nc.scalar.dma_start(xt[0][:, 0:256], xT_d[0:P, 0:256])
            nc.scalar.dma_start(qt[1][:, 0:512], qT_d[hs1, 0:512])
            nc.scalar.dma_start(xt[1][:, 0:256], xT_d[hs1, 0:256])
            nc.scalar.dma_start(wsc[:], wsc_d[:])
            nc.scalar.dma_start(vt[0][:], v_d[0:P, :])
            nc.scalar.dma_start(vt[1][:], v_d[P:2 * P, :])
            # everything else on the sync ring, in need-order
            ld_cols(qt, qT_d, slice(512, 1024))
            ld_cols(xt, xT_d, slice(256, 512))
            ld_v(2)
            ld_v(3)
            ld_cols(xt, xT_d, slice(512, 1024))
            ld_v(4)
            ld_v(5)
            ld_cols(xt, xT_d, slice(1024, 2048))
            for j in range(6, 10):
                ld_v(j)
            ld_cols(xt, xT_d, slice(2048, 3072))
            for j in range(10, 18):
                ld_v(j)
            ld_cols(xt, xT_d, slice(3072, 4096))
            for j in range(18, 26):
                ld_v(j)
            ld_cols(qt, qT_d, slice(1024, 2048))
            for j in range(26, 32):
                ld_v(j)

            # ---- PE warm-up + ACT table preload (no DMA dependencies;
            # gpsimd memset because that engine comes alive first) ----
            wrm = io.tile([P, 256], FR, tag="wrm", name="wrm")
            nc.gpsimd.memset(wrm[:], 0.5)
            wex = op.tile([P, 2], f32, tag="wex", name="wex")
            nc.scalar.activation(wex[:], wrm[:, 0:2], Exp)
            wps = psmm.tile([P, 256], f32, tag="mm", name="wps")
            nwarm = 16 + salt
            for r in range(nwarm):
                nc.tensor.matmul(wps[:, 0:P], wrm[:, 0:P], wrm[:, 0:P],
                                 start=(r == 0), stop=(r == nwarm - 1))
            # read wps out immediately: its PSUM buffer (tag "mm") is
            # recycled by the scores tiles below.  The flush keeps the
            # junk matmuls + dummy exp alive through DCE.
            wsb = op.tile([P, 4], f32, tag="wsb", name="wsb")
            nc.vector.tensor_copy(wsb[:, 0:2], wps[:, 0:2])
            nc.vector.tensor_copy(wsb[:, 2:4], wex[:])
            nc.sync.dma_start(warm_d[:], wsb[:])

            # ---- attention block pairs ----
            # The scores matmuls for both blocks of a pair share each
            # key-chunk stationary load.  Block b0's AV runs inline per
            # key-chunk; block b1's exp(S) tiles are buffered in SBUF and
            # consumed in a second AV sweep (PSUM can only hold one
            # block's accumulators plus the rotating scores tiles).
            def normalize_one(att_tile, blk, ic):
                rec = op.tile([P, 1], f32, tag="rec", name="rec")
                nc.vector.reciprocal(rec[:], att_tile[:, H:H + 1])
                ao = op.tile([P, H], AVT, tag="ao", name="ao")
                nc.vector.tensor_scalar_mul(ao[:], att_tile[:, 0:H], rec[:])
                r0 = blk * IBLK + ic * P
                nc.sync.dma_start(att_d[r0:r0 + P, :], ao[:])

            for pair in range(NPAIR):
                bss = [slice((2 * pair + b) * IBLK, (2 * pair + b + 1) * IBLK)
                       for b in range(2)]
                att_ps = [psatt.tile([P, H + 2], f32, tag="att", name="attps")
                          for _ in range(ICH)]
                exs = [[], []]
                for jc in range(JC):
                    jcs = slice(jc * P, (jc + 1) * P)
                    scs = [psmm.tile([P, IBLK], f32, tag="mm", name=f"sc{b}")
                           for b in range(2)]
                    if pair == 0 and jc < 4:
                        # block-major: b0 only needs the first qt chunks,
                        # which land ~1us before b1's
                        for b in range(2):
                            for oc in range(2):
                                nc.tensor.matmul(scs[b][:], xt[oc][:, jcs],
                                                 qt[oc][:, bss[b]],
                                                 start=(oc == 0), stop=(oc == 1))
                    else:
                        for oc in range(2):
                            for b in range(2):
                                nc.tensor.matmul(scs[b][:], xt[oc][:, jcs],
                                                 qt[oc][:, bss[b]],
                                                 start=(oc == 0), stop=(oc == 1))
                    for b in range(2):
                        ex = expp.tile([P, IBLK], AVT, tag="ex", name=f"ex{b}")
                        nc.scalar.activation(ex[:], scs[b][:], Exp,
                                             bias=wsc[:, jc:jc + 1])
                        exs[b].append(ex)
                    for ic in range(ICH):
                        ics = slice(ic * P, (ic + 1) * P)
                        nc.tensor.matmul(att_ps[ic][:], exs[0][jc][:, ics],
                                         vt[jc][:],
                                         start=(jc == 0), stop=(jc == JC - 1))
                for ic in range(ICH):
                    normalize_one(att_ps[ic], 2 * pair, ic)
                last = (pair == NPAIR - 1)
                for ic in range(ICH):
                    ics = slice(ic * P, (ic + 1) * P)
                    if last and ic == ICH - 1:
                        # final accumulator: split by V columns across two
                        # PSUM banks so the high half's normalize + DMA-out
                        # overlaps the low half's AV sweep.
                        pa = psatt.tile([P, H - P + 2], f32, tag="att",
                                        name="attpa")
                        pb = psatt.tile([P, P], f32, tag="att", name="attpb")
                        for jc in range(JC):
                            nc.tensor.matmul(pa[:], exs[1][jc][:, ics],
                                             vt[jc][:, P:H + 2],
                                             start=(jc == 0),
                                             stop=(jc == JC - 1))
                        rec = op.tile([P, 1], f32, tag="rec", name="rec")
                        nc.vector.reciprocal(rec[:], pa[:, H - P:H - P + 1])
                        ah = op.tile([P, H - P], AVT, tag="ao", name="ah")
                        nc.vector.tensor_scalar_mul(ah[:], pa[:, 0:H - P],
                                                    rec[:])
                        r0 = (2 * pair + 1) * IBLK + ic * P
                        nc.sync.dma_start(att_d[r0:r0 + P, P:H], ah[:])
                        for jc in range(JC):
                            nc.tensor.matmul(pb[:], exs[1][jc][:, ics],
                                             vt[jc][:, 0:P],
                                             start=(jc == 0),
                                             stop=(jc == JC - 1))
                        al = op.tile([P, P], AVT, tag="ao", name="al")
                        nc.vector.tensor_scalar_mul(al[:], pb[:], rec[:])
                        nc.sync.dma_start(att_d[r0:r0 + P, 0:P], al[:])
                    else:
                        pf = psatt.tile([P, H + 2], f32, tag="att",
                                        name="attpsb")
                        for jc in range(JC):
                            nc.tensor.matmul(pf[:], exs[1][jc][:, ics],
                                             vt[jc][:],
                                             start=(jc == 0),
                                             stop=(jc == JC - 1))
                        normalize_one(pf, 2 * pair + 1, ic)

    nc.compile()
    return nc


_NC_CACHE = {}


def _get_nc():
    if "nc" not in _NC_CACHE:
        _NC_CACHE["nc"] = build_nc()
    return _NC_CACHE["nc"]


def _make_in_maps(x, Wq, bq, Wk, bk, Wv):
    import ml_dtypes

    bf16 = ml_dtypes.bfloat16
    A = Wq.T.astype(np.float64) @ Wk.astype(np.float64)
    wkbq = Wk.T.astype(np.float64) @ bq.astype(np.float64)
    in_maps = []
    for b in range(B):
        xb = x[b].astype(np.float64)
        wsc_b = np.ascontiguousarray(
            (xb @ wkbq).astype(np.float32).reshape(JC, P).T)
        v_b = np.empty((N, H + 2), dtype=bf16)
        v_b[:, 0:H] = (xb @ Wv.T.astype(np.float64)).astype(bf16)
        v_b[:, H:] = np.ones((N, 2), dtype=bf16)
        v_b = np.ascontiguousarray(v_b)
        xT_b = np.ascontiguousarray(x[b].astype(np.float16).T)
        q_b = (xb @ A).astype(np.float16)
        for half in range(2):
            qT = np.ascontiguousarray(q_b[half * NQ:(half + 1) * NQ, :].T)
            in_maps.append({"xT": xT_b, "qT": qT, "v": v_b, "wsc": wsc_b})
    return in_maps


def _run(inputs, trace=False):
    from concourse.bass_utils import run_bass_kernel_spmd

    x = np.asarray(inputs["x"], dtype=np.float32)
    Wq = np.asarray(inputs["Wq"], dtype=np.float32)
    bq = np.asarray(inputs["bq"], dtype=np.float32)
    Wk = np.asarray(inputs["Wk"], dtype=np.float32)
    bk = np.asarray(inputs["bk"], dtype=np.float32)
    Wv = np.asarray(inputs["Wv"], dtype=np.float32)
    bv = np.asarray(inputs["bv"], dtype=np.float32)

    in_maps = _make_in_maps(x, Wq, bq, Wk, bk, Wv)
    # The device occasionally wedges on the first execution of a fresh
    # NEFF (NRT_EXEC_UNIT_UNRECOVERABLE); a retry with a slightly
    # perturbed program (different walrus schedule) recovers.
    last_exc = None
    for attempt in range(3):
        try:
            nc = _get_nc() if attempt == 0 else build_nc(salt=attempt)
            res = run_bass_kernel_spmd(nc, in_maps, list(range(8)), trace=trace)
            break
        except Exception as e:  # noqa: BLE001
            last_exc = e
            import os as _os
            import time as _time
            _os.environ["NEURON_RT_RESET_CORES"] = "1"
            _time.sleep(3)
    else:
        raise last_exc

    out = np.empty((B, N, H), dtype=np.float32)
    for c in range(8):
        b, half = c // 2, c % 2
        out[b, half * NQ:(half + 1) * NQ, :] = \
            res.results[c]["att"].astype(np.float32) + bv
    return out, res


def kernel(**inputs) -> np.ndarray:
    out, _ = _run(inputs, trace=False)
    return out
